# revision 1
# baseline (speedup 1.0000x reference)
"""Trainium2 Bass kernel for the CellularAutomata step (dense_cnn).

Math (per pixel): s = depthwise3x3(wrap_pad(x), [identity, sobel_x, sobel_y]);
h = relu(s @ W1 + b1); out = clip(x + h @ W2 + b2, 0, 1).

Strategy (pure data parallel, batch -> 8 cores, weights replicated):
  - Host: per-core image to channel-major flat layout [16, 258*258] with wrap
    padding; the whole output (with junk wrap columns) is computed on a padded
    flat grid and the host slices out the valid 256x256 region.
  - The 3x3 perception conv + W1 are folded (host-side) into three [48, 128]
    matrices, one per vertical tap dy.  The device loads x three times at flat
    offsets +0/+1/+2 onto partition blocks 0-15/16-31/32-47, so each dy is a
    single K=48 matmul whose free-dim offset dy*258 walks the rows; the three
    dy matmuls accumulate in PSUM.  float32r -> full PE rate at N=512.
  - The residual "+x" is an extra K=16 identity matmul into the dx PSUM.
    dx for 4 consecutive 512-pixel chunks is stacked at PSUM partition strips
    0/32/64/96 (explicit tile_position), so bias+clip post-ops run on 128
    partitions on the DVE, then one DMA writes all 4 chunks out.
"""

import numpy as np
from contextlib import ExitStack

import concourse.bass as bass
import concourse.tile as tile
from concourse import bacc, mybir
from concourse.bass_utils import run_bass_kernel_spmd

B, S, C, HID = 8, 256, 16, 128
N_CORES = 8
P = S + 2                    # padded width = 258
FLAT = P * P                 # 66564
CH = 512                     # pixels per chunk
BF = 4                       # chunks per block (shared DMA)
NCHUNK = 130                 # covers all valid padded-flat positions
NB = (NCHUNK + BF - 1) // BF
SPAN = (BF - 1) * CH + CH + 2 * P + 8   # block free extent read by matmuls
XLEN = 144 * CH + 2 * P + 16            # padded flat x length (covers bf<=16)

_CACHE = {}


def _build_program(bf=BF, xx_bufs=6, h_bufs=4, u_bufs=3, o_bufs=4,
                   ph_bufs=2, pdx_bufs=4, repeat=1, use_gpsimd=False, mode='full',
                   use_bf16=False, act_every=1000, relu_mode='act', probe=None):
    f32 = mybir.dt.float32
    f32r = mybir.dt.float32r
    Relu = mybir.ActivationFunctionType.Relu
    add = mybir.AluOpType.add
    op_max = mybir.AluOpType.max
    op_min = mybir.AluOpType.min

    nc = bacc.Bacc("TRN2", target_bir_lowering=False, debug=False,
                   num_devices=N_CORES)

    bf16 = mybir.dt.bfloat16
    mdt = bf16 if use_bf16 else f32r
    xf = nc.dram_tensor("xf", [C, XLEN], f32r, kind="ExternalInput").ap()
    xfb = (nc.dram_tensor("xfb", [C, XLEN], bf16, kind="ExternalInput").ap()
           if use_bf16 else None)
    wc = nc.dram_tensor("wc", [48, 3 * HID], mdt, kind="ExternalInput").ap()
    w2 = nc.dram_tensor("w2", [HID, 32], mdt, kind="ExternalInput").ap()
    b1 = nc.dram_tensor("b1", [HID, 1], f32, kind="ExternalInput").ap()
    b2s = nc.dram_tensor("b2s", [C, 1], f32, kind="ExternalInput").ap()
    out = nc.dram_tensor("out", [C, XLEN], f32, kind="ExternalOutput").ap()

    with tile.TileContext(nc) as tc, ExitStack() as ctx:
        wpool = ctx.enter_context(tc.tile_pool(name="wts", bufs=1))
        wc_sb = wpool.tile([48, 3 * HID], mdt)
        nc.sync.dma_start(wc_sb[:], wc)
        w2_sb = wpool.tile([HID, 32], mdt)
        nc.sync.dma_start(w2_sb[:], w2)
        b1_sb = wpool.tile([HID, 1], f32)
        nc.sync.dma_start(b1_sb[:], b1)
        b2_sb = wpool.tile([C, 1], f32)
        nc.sync.dma_start(b2_sb[:], b2s)

        xpool = ctx.enter_context(tc.tile_pool(name="xx", bufs=xx_bufs))
        xcpool = ctx.enter_context(tc.tile_pool(name="xc", bufs=xx_bufs))
        hpool = ctx.enter_context(tc.tile_pool(name="h", bufs=h_bufs))
        upool = ctx.enter_context(tc.tile_pool(name="u", bufs=u_bufs))
        opool = ctx.enter_context(tc.tile_pool(name="o", bufs=o_bufs))
        ph_pool = ctx.enter_context(tc.tile_pool(name="ph", bufs=ph_bufs, space="PSUM"))
        pdx_pool = ctx.enter_context(tc.tile_pool(name="pdx", bufs=pdx_bufs, space="PSUM"))

        nblocks = (NCHUNK + bf - 1) // bf
        span = (bf - 1) * CH + CH + 2 * P + 8
        rep_cm = tc.For_i(0, repeat, 1) if repeat > 1 else None
        if rep_cm is not None:
            rep_cm.__enter__()

        def load_xx(b):
            # x triple in ONE DMA: overlapping-window source AP
            # dst partition 16*dc+cc <- xsrc[cc, b*bf*CH + dc + f]
            q0 = b * bf * CH
            xsrc = xfb if use_bf16 else xf
            xx = xpool.tile([48, span], mdt, tag="xx")
            base = xsrc[:, q0:q0 + span]
            src = bass.AP(tensor=base.tensor, offset=base.offset,
                          ap=[[1, 3]] + [list(p) for p in base.ap])
            nc.sync.dma_start(xx[:], src)
            xc = None
            if use_bf16:
                # exact f32 center strip for the residual add
                xc = xcpool.tile([C, bf * CH], f32r, tag="xc")
                nc.sync.dma_start(xc[:], xf[:, q0 + P + 1:q0 + P + 1 + bf * CH])
            return xx, xc

        xx, xc = load_xx(0)
        pending_out = None   # (o_tile, p0) emitted one block later
        for b in range(nblocks):
            p0 = b * bf * CH
            xx_next, xc_next = load_xx(b + 1) if b + 1 < nblocks else (None, None)
            if pending_out is not None:
                po, pp0 = pending_out
                nc.sync.dma_start(
                    out[:, pp0 + P + 1:pp0 + P + 1 + bf * CH], po[:])

            o = opool.tile([C, bf * CH], f32)
            hs = []
            # phase 1: all tap matmuls (PE) + relu (ACT) — keeps the PE
            # queue free of instructions that wait on other engines.
            # chunks are processed in pairs sharing a 2-bank PSUM tile so
            # one ACT relu covers 1024 columns.
            for sp in range(bf // 2):
                ph = ph_pool.tile([HID, 2 * CH], f32)
                ntap = 1 if probe == 'taps1' else 3
                for s2 in range(2):
                    f0 = (2 * sp + s2) * CH
                    for dy in range(ntap):
                        nc.tensor.matmul(
                            ph[:, s2 * CH:(s2 + 1) * CH],
                            lhsT=wc_sb[:, dy * HID:(dy + 1) * HID],
                            rhs=xx[:, f0 + dy * P:f0 + dy * P + CH],
                            start=(dy == 0), stop=(dy == ntap - 1),
                        )
                h = hpool.tile([HID, 2 * CH], mdt)
                nc.scalar.activation(h[:], ph[:], Relu, bias=b1_sb[:])
                hs.append(h)
            # phase 2: MLP2 matmuls (PE), then post-ops (DVE)
            pdxs = []
            for s in range(bf):
                pdx = pdx_pool.tile([32, CH], f32)
                nc.tensor.matmul(pdx[:], lhsT=w2_sb[:],
                                 rhs=hs[s // 2][:, (s % 2) * CH:(s % 2 + 1) * CH],
                                 start=True, stop=True)
                pdxs.append(pdx)
            if mode == 'full':
                # u = dx + x_center per chunk, then block-wide
                # o = min(max(u + b2, 0), 1) in two fused DVE ops
                u = upool.tile([C, bf * CH], f32)
                for s in range(bf):
                    f0 = s * CH
                    if use_bf16:
                        res_src = xc[:, f0:f0 + CH].bitcast(f32)
                    else:
                        res_src = xx[0:C, f0 + P + 1:f0 + P + 1 + CH].bitcast(f32)
                    if probe == 'nott':
                        nc.vector.tensor_copy(u[:, f0:f0 + CH], pdxs[s][0:C, :])
                    else:
                        nc.vector.tensor_tensor(
                            u[:, f0:f0 + CH], pdxs[s][0:C, :], res_src, op=add)
                nc.vector.tensor_scalar(u[:], u[:], b2_sb[:], 0.0,
                                        op0=add, op1=op_max)
                nc.vector.tensor_scalar_min(o[:], u[:], 1.0)

            pending_out = (o, p0)
            xx, xc = xx_next, xc_next
        po, pp0 = pending_out
        nc.sync.dma_start(out[:, pp0 + P + 1:pp0 + P + 1 + bf * CH], po[:])
        if rep_cm is not None:
            rep_cm.__exit__(None, None, None)

    nc.compile()
    return nc


def _prep_weights(pk, W1):
    # pk [3(dy),3(dx),3(k)]; W1 [48,128] rows indexed 3*ci+k
    W1r = W1.reshape(C, 3, HID)                      # [ci, k, hid]
    Wfull = np.einsum("ydk,ckh->ydch", pk, W1r)      # [dy, dx, ci, hid]
    return np.ascontiguousarray(
        np.concatenate([Wfull[0].reshape(3 * C, HID),
                        Wfull[1].reshape(3 * C, HID),
                        Wfull[2].reshape(3 * C, HID)], axis=1), dtype=np.float32)


def kernel(x, perception_kernel, W1, b1, W2, b2):
    x = np.asarray(x, dtype=np.float32)
    pk = np.asarray(perception_kernel, dtype=np.float32)
    W1 = np.asarray(W1, dtype=np.float32)
    b1 = np.asarray(b1, dtype=np.float32)
    W2 = np.asarray(W2, dtype=np.float32)
    b2 = np.asarray(b2, dtype=np.float32)

    if "nc" not in _CACHE:
        _CACHE["nc"] = _build_program()
    nc = _CACHE["nc"]

    wc_np = _prep_weights(pk, W1)
    w2_np = np.zeros((HID, 32), np.float32)
    w2_np[:, :C] = W2
    b1_np = np.ascontiguousarray(b1.reshape(HID, 1))
    b2s = np.ascontiguousarray(b2.reshape(C, 1))

    in_maps = []
    for c in range(N_CORES):
        xt = np.ascontiguousarray(x[c].transpose(2, 0, 1))      # [C, S, S]
        xt = np.pad(xt, ((0, 0), (1, 1), (1, 1)), mode="wrap")  # [C, 258, 258]
        xflat = np.zeros((C, XLEN), np.float32)
        xflat[:, :FLAT] = xt.reshape(C, FLAT)
        in_maps.append({
            "xf": xflat, "wc": wc_np, "w2": w2_np,
            "b1": b1_np, "b2s": b2s,
        })

    res = run_bass_kernel_spmd(nc, in_maps, list(range(N_CORES)))
    _CACHE["exec_time_ns"] = getattr(res, "exec_time_ns", None)
    _CACHE["trace"] = getattr(res, "instructions_and_trace", None)
    outs = []
    for c in range(N_CORES):
        of = res.results[c]["out"][:, :FLAT].reshape(C, P, P)
        outs.append(of[:, 1:S + 1, 1:S + 1].transpose(1, 2, 0))
    return np.ascontiguousarray(np.stack(outs, axis=0), dtype=np.float32)



# revision 2
# speedup vs baseline: 1.1312x; 1.1312x over previous
"""Trainium2 Bass kernel for the CellularAutomata step (dense_cnn).

Math (per pixel): s = depthwise3x3(wrap_pad(x), [identity, sobel_x, sobel_y]);
h = relu(s @ W1 + b1); out = clip(x + h @ W2 + b2, 0, 1).

Strategy (pure data parallel, batch -> 8 cores, weights replicated):
  - Host: per-core image to channel-major flat layout [16, 258*258] with wrap
    padding; the whole output (with junk wrap columns) is computed on a padded
    flat grid and the host slices out the valid 256x256 region.
  - The 3x3 perception conv + W1 are folded (host-side) into three [48, 128]
    matrices, one per vertical tap dy.  The device loads x three times at flat
    offsets +0/+1/+2 onto partition blocks 0-15/16-31/32-47, so each dy is a
    single K=48 matmul whose free-dim offset dy*258 walks the rows; the three
    dy matmuls accumulate in PSUM.  float32r -> full PE rate at N=512.
  - The residual "+x" is an extra K=16 identity matmul into the dx PSUM.
    dx for 4 consecutive 512-pixel chunks is stacked at PSUM partition strips
    0/32/64/96 (explicit tile_position), so bias+clip post-ops run on 128
    partitions on the DVE, then one DMA writes all 4 chunks out.
"""

import numpy as np
from contextlib import ExitStack

import concourse.bass as bass
import concourse.tile as tile
from concourse import bacc, mybir
from concourse.bass_utils import run_bass_kernel_spmd

B, S, C, HID = 8, 256, 16, 128
N_CORES = 8
P = S + 2                    # padded width = 258
FLAT = P * P                 # 66564
CH = 512                     # pixels per chunk
BF = 4                       # chunks per block (shared DMA)
NCHUNK = 130                 # covers all valid padded-flat positions
NB = (NCHUNK + BF - 1) // BF
SPAN = (BF - 1) * CH + CH + 2 * P + 8   # block free extent read by matmuls
XLEN = 144 * CH + 2 * P + 16            # padded flat x length (covers bf<=16)

_CACHE = {}


def _build_program(bf=BF, xx_bufs=6, h_bufs=4, u_bufs=3, o_bufs=4,
                   ph_bufs=2, pdx_bufs=4, repeat=1, use_gpsimd=False, mode='full',
                   use_bf16=False, act_every=1000, relu_mode='act', probe=None):
    f32 = mybir.dt.float32
    f32r = mybir.dt.float32r
    Relu = mybir.ActivationFunctionType.Relu
    add = mybir.AluOpType.add
    op_max = mybir.AluOpType.max
    op_min = mybir.AluOpType.min

    nc = bacc.Bacc("TRN2", target_bir_lowering=False, debug=False,
                   num_devices=N_CORES)

    bf16 = mybir.dt.bfloat16
    mdt = bf16 if use_bf16 else f32r
    xf = nc.dram_tensor("xf", [C, XLEN], f32r, kind="ExternalInput").ap()
    xfb = (nc.dram_tensor("xfb", [C, XLEN], bf16, kind="ExternalInput").ap()
           if use_bf16 else None)
    wc = nc.dram_tensor("wc", [48, 3 * HID], mdt, kind="ExternalInput").ap()
    w2 = nc.dram_tensor("w2", [HID, 32], mdt, kind="ExternalInput").ap()
    b1 = nc.dram_tensor("b1", [HID, 1], f32, kind="ExternalInput").ap()
    b2s = nc.dram_tensor("b2s", [C, 1], f32, kind="ExternalInput").ap()
    out = nc.dram_tensor("out", [C, XLEN], f32, kind="ExternalOutput").ap()

    with tile.TileContext(nc) as tc, ExitStack() as ctx:
        wpool = ctx.enter_context(tc.tile_pool(name="wts", bufs=1))
        wc_sb = wpool.tile([48, 3 * HID], mdt)
        nc.sync.dma_start(wc_sb[:], wc)
        w2_sb = wpool.tile([HID, 32], mdt)
        nc.sync.dma_start(w2_sb[:], w2)
        b1_sb = wpool.tile([HID, 1], f32)
        nc.sync.dma_start(b1_sb[:], b1)
        b2_sb = wpool.tile([C, 1], f32)
        nc.sync.dma_start(b2_sb[:], b2s)

        xpool = ctx.enter_context(tc.tile_pool(name="xx", bufs=xx_bufs))
        xcpool = ctx.enter_context(tc.tile_pool(name="xc", bufs=xx_bufs))
        hpool = ctx.enter_context(tc.tile_pool(name="h", bufs=h_bufs))
        upool = ctx.enter_context(tc.tile_pool(name="u", bufs=u_bufs))
        opool = ctx.enter_context(tc.tile_pool(name="o", bufs=o_bufs))
        ph_pool = ctx.enter_context(tc.tile_pool(name="ph", bufs=ph_bufs, space="PSUM"))
        pdx_pool = ctx.enter_context(tc.tile_pool(name="pdx", bufs=pdx_bufs, space="PSUM"))

        nblocks = (NCHUNK + bf - 1) // bf
        span = (bf - 1) * CH + CH + 2 * P + 8
        rep_cm = tc.For_i(0, repeat, 1) if repeat > 1 else None
        if rep_cm is not None:
            rep_cm.__enter__()

        def load_xx(b):
            # x triple as THREE per-dx DMAs: each src AP has outer dim 16
            # (channels), so the HWDGE spreads descriptors over all 16 DMA
            # engines instead of 3 (engine choice follows the outermost AP
            # dim index).  dst partition 16*dc+cc <- xsrc[cc, b*bf*CH+dc+f]
            q0 = b * bf * CH
            xsrc = xfb if use_bf16 else xf
            xx = xpool.tile([48, span], mdt, tag="xx")
            for dc in range(3):
                nc.sync.dma_start(xx[16 * dc:16 * (dc + 1), :],
                                  xsrc[:, q0 + dc:q0 + dc + span])
            xc = None
            if use_bf16:
                # exact f32 center strip for the residual add
                xc = xcpool.tile([C, bf * CH], f32r, tag="xc")
                nc.sync.dma_start(xc[:], xf[:, q0 + P + 1:q0 + P + 1 + bf * CH])
            return xx, xc

        xx, xc = load_xx(0)
        pending_out = None   # (o_tile, p0) emitted one block later
        for b in range(nblocks):
            p0 = b * bf * CH
            xx_next, xc_next = load_xx(b + 1) if b + 1 < nblocks else (None, None)
            if pending_out is not None:
                po, pp0 = pending_out
                nc.sync.dma_start(
                    out[:, pp0 + P + 1:pp0 + P + 1 + bf * CH], po[:])

            o = opool.tile([C, bf * CH], f32)
            hs = []
            # phase 1: all tap matmuls (PE) + relu (ACT) — keeps the PE
            # queue free of instructions that wait on other engines.
            # chunks are processed in pairs sharing a 2-bank PSUM tile so
            # one ACT relu covers 1024 columns.
            for sp in range(bf // 2):
                ph = ph_pool.tile([HID, 2 * CH], f32)
                ntap = 1 if probe == 'taps1' else 3
                for s2 in range(2):
                    f0 = (2 * sp + s2) * CH
                    for dy in range(ntap):
                        nc.tensor.matmul(
                            ph[:, s2 * CH:(s2 + 1) * CH],
                            lhsT=wc_sb[:, dy * HID:(dy + 1) * HID],
                            rhs=xx[:, f0 + dy * P:f0 + dy * P + CH],
                            start=(dy == 0), stop=(dy == ntap - 1),
                        )
                h = hpool.tile([HID, 2 * CH], mdt)
                nc.scalar.activation(h[:], ph[:], Relu, bias=b1_sb[:])
                hs.append(h)
            # phase 2: MLP2 matmuls (PE), then post-ops (DVE)
            pdxs = []
            for s in range(bf):
                pdx = pdx_pool.tile([32, CH], f32)
                nc.tensor.matmul(pdx[:], lhsT=w2_sb[:],
                                 rhs=hs[s // 2][:, (s % 2) * CH:(s % 2 + 1) * CH],
                                 start=True, stop=True)
                pdxs.append(pdx)
            if mode == 'full':
                # u = dx + x_center per chunk, then block-wide
                # o = min(max(u + b2, 0), 1) in two fused DVE ops
                u = upool.tile([C, bf * CH], f32)
                for s in range(bf):
                    f0 = s * CH
                    if use_bf16:
                        res_src = xc[:, f0:f0 + CH].bitcast(f32)
                    else:
                        res_src = xx[0:C, f0 + P + 1:f0 + P + 1 + CH].bitcast(f32)
                    if probe == 'nott':
                        nc.vector.tensor_copy(u[:, f0:f0 + CH], pdxs[s][0:C, :])
                    else:
                        nc.vector.tensor_tensor(
                            u[:, f0:f0 + CH], pdxs[s][0:C, :], res_src, op=add)
                nc.vector.tensor_scalar(u[:], u[:], b2_sb[:], 0.0,
                                        op0=add, op1=op_max)
                nc.vector.tensor_scalar_min(o[:], u[:], 1.0)

            pending_out = (o, p0)
            xx, xc = xx_next, xc_next
        po, pp0 = pending_out
        nc.sync.dma_start(out[:, pp0 + P + 1:pp0 + P + 1 + bf * CH], po[:])
        if rep_cm is not None:
            rep_cm.__exit__(None, None, None)

    nc.compile()
    return nc


def _prep_weights(pk, W1):
    # pk [3(dy),3(dx),3(k)]; W1 [48,128] rows indexed 3*ci+k
    W1r = W1.reshape(C, 3, HID)                      # [ci, k, hid]
    Wfull = np.einsum("ydk,ckh->ydch", pk, W1r)      # [dy, dx, ci, hid]
    return np.ascontiguousarray(
        np.concatenate([Wfull[0].reshape(3 * C, HID),
                        Wfull[1].reshape(3 * C, HID),
                        Wfull[2].reshape(3 * C, HID)], axis=1), dtype=np.float32)


def kernel(x, perception_kernel, W1, b1, W2, b2):
    x = np.asarray(x, dtype=np.float32)
    pk = np.asarray(perception_kernel, dtype=np.float32)
    W1 = np.asarray(W1, dtype=np.float32)
    b1 = np.asarray(b1, dtype=np.float32)
    W2 = np.asarray(W2, dtype=np.float32)
    b2 = np.asarray(b2, dtype=np.float32)

    if "nc" not in _CACHE:
        _CACHE["nc"] = _build_program()
    nc = _CACHE["nc"]

    wc_np = _prep_weights(pk, W1)
    w2_np = np.zeros((HID, 32), np.float32)
    w2_np[:, :C] = W2
    b1_np = np.ascontiguousarray(b1.reshape(HID, 1))
    b2s = np.ascontiguousarray(b2.reshape(C, 1))

    in_maps = []
    for c in range(N_CORES):
        xt = np.ascontiguousarray(x[c].transpose(2, 0, 1))      # [C, S, S]
        xt = np.pad(xt, ((0, 0), (1, 1), (1, 1)), mode="wrap")  # [C, 258, 258]
        xflat = np.zeros((C, XLEN), np.float32)
        xflat[:, :FLAT] = xt.reshape(C, FLAT)
        in_maps.append({
            "xf": xflat, "wc": wc_np, "w2": w2_np,
            "b1": b1_np, "b2s": b2s,
        })

    res = run_bass_kernel_spmd(nc, in_maps, list(range(N_CORES)))
    _CACHE["exec_time_ns"] = getattr(res, "exec_time_ns", None)
    _CACHE["trace"] = getattr(res, "instructions_and_trace", None)
    outs = []
    for c in range(N_CORES):
        of = res.results[c]["out"][:, :FLAT].reshape(C, P, P)
        outs.append(of[:, 1:S + 1, 1:S + 1].transpose(1, 2, 0))
    return np.ascontiguousarray(np.stack(outs, axis=0), dtype=np.float32)



# revision 4
# speedup vs baseline: 1.1816x; 1.0445x over previous
"""Trainium2 Bass kernel for the CellularAutomata step (dense_cnn).

Math (per pixel): s = depthwise3x3(wrap_pad(x), [identity, sobel_x, sobel_y]);
h = relu(s @ W1 + b1); out = clip(x + h @ W2 + b2, 0, 1).

Strategy (pure data parallel, batch -> 8 cores, weights replicated):
  - Host: per-core image to channel-major flat layout [16, 258*258] with wrap
    padding; the whole output (with junk wrap columns) is computed on a padded
    flat grid and the host slices out the valid 256x256 region.
  - The 3x3 perception conv + W1 are folded (host-side) into three [48, 128]
    matrices, one per vertical tap dy.  The device loads x three times at flat
    offsets +0/+1/+2 onto partition blocks 0-15/16-31/32-47, so each dy is a
    single K=48 matmul whose free-dim offset dy*258 walks the rows; the three
    dy matmuls accumulate in PSUM.  float32r -> full PE rate at N=512.
  - The residual "+x" is an extra K=16 identity matmul into the dx PSUM.
    dx for 4 consecutive 512-pixel chunks is stacked at PSUM partition strips
    0/32/64/96 (explicit tile_position), so bias+clip post-ops run on 128
    partitions on the DVE, then one DMA writes all 4 chunks out.
"""

import numpy as np
from contextlib import ExitStack

import concourse.bass as bass
import concourse.tile as tile
from concourse import bacc, mybir
from concourse.bass_utils import run_bass_kernel_spmd

B, S, C, HID = 8, 256, 16, 128
N_CORES = 8
P = S + 2                    # padded width = 258
FLAT = P * P                 # 66564
CH = 512                     # pixels per chunk
BF = 4                       # chunks per block (shared DMA)
NCHUNK = 130                 # covers all valid padded-flat positions
NB = (NCHUNK + BF - 1) // BF
SPAN = (BF - 1) * CH + CH + 2 * P + 8   # block free extent read by matmuls
XLEN = 144 * CH + 2 * P + 16            # padded flat x length (covers bf<=16)

_CACHE = {}


def _build_program(bf=BF, xx_bufs=6, h_bufs=4, u_bufs=3, o_bufs=4,
                   ph_bufs=2, pdx_bufs=4, repeat=1, use_gpsimd=False, mode='full',
                   use_bf16=False, act_every=1000, relu_mode='act', probe=None):
    f32 = mybir.dt.float32
    f32r = mybir.dt.float32r
    Relu = mybir.ActivationFunctionType.Relu
    add = mybir.AluOpType.add
    op_max = mybir.AluOpType.max
    op_min = mybir.AluOpType.min

    nc = bacc.Bacc("TRN2", target_bir_lowering=False, debug=False,
                   num_devices=N_CORES)

    bf16 = mybir.dt.bfloat16
    mdt = bf16 if use_bf16 else f32r
    xf = nc.dram_tensor("xf", [C, XLEN], f32r, kind="ExternalInput").ap()
    xfb = (nc.dram_tensor("xfb", [C, XLEN], bf16, kind="ExternalInput").ap()
           if use_bf16 else None)
    wc = nc.dram_tensor("wc", [48, 3 * HID], mdt, kind="ExternalInput").ap()
    w2 = nc.dram_tensor("w2", [HID, 32], mdt, kind="ExternalInput").ap()
    b1 = nc.dram_tensor("b1", [HID, 1], f32, kind="ExternalInput").ap()
    b2s = nc.dram_tensor("b2s", [C, 1], f32, kind="ExternalInput").ap()
    out = nc.dram_tensor("out", [C, XLEN], f32, kind="ExternalOutput").ap()

    with tile.TileContext(nc) as tc, ExitStack() as ctx:
        wpool = ctx.enter_context(tc.tile_pool(name="wts", bufs=1))
        wc_sb = wpool.tile([48, 3 * HID], mdt)
        nc.sync.dma_start(wc_sb[:], wc)
        w2_sb = wpool.tile([HID, 32], mdt)
        nc.sync.dma_start(w2_sb[:], w2)
        b1_sb = wpool.tile([HID, 1], f32)
        nc.sync.dma_start(b1_sb[:], b1)
        b2_sb = wpool.tile([C, 1], f32)
        nc.sync.dma_start(b2_sb[:], b2s)

        xpool = ctx.enter_context(tc.tile_pool(name="xx", bufs=xx_bufs))
        xcpool = ctx.enter_context(tc.tile_pool(name="xc", bufs=xx_bufs))
        hpool = ctx.enter_context(tc.tile_pool(name="h", bufs=h_bufs))
        upool = ctx.enter_context(tc.tile_pool(name="u", bufs=u_bufs))
        opool = ctx.enter_context(tc.tile_pool(name="o", bufs=o_bufs))
        ph_pool = ctx.enter_context(tc.tile_pool(name="ph", bufs=ph_bufs, space="PSUM"))
        pdx_pool = ctx.enter_context(tc.tile_pool(name="pdx", bufs=pdx_bufs, space="PSUM"))

        nblocks = (NCHUNK + bf - 1) // bf
        span = (bf - 1) * CH + CH + 2 * P + 8
        rep_cm = tc.For_i(0, repeat, 1) if repeat > 1 else None
        if rep_cm is not None:
            rep_cm.__enter__()

        def load_xx(b):
            # x triple as THREE per-dx DMAs: each src AP has outer dim 16
            # (channels), so the HWDGE spreads descriptors over all 16 DMA
            # engines instead of 3 (engine choice follows the outermost AP
            # dim index).  dst partition 16*dc+cc <- xsrc[cc, b*bf*CH+dc+f]
            q0 = b * bf * CH
            xsrc = xfb if use_bf16 else xf
            xx = xpool.tile([48, span], mdt, tag="xx")
            for dc in range(3):
                nc.sync.dma_start(xx[16 * dc:16 * (dc + 1), :],
                                  xsrc[:, q0 + dc:q0 + dc + span])
            xc = None
            if use_bf16:
                # exact f32 center strip for the residual add
                xc = xcpool.tile([C, bf * CH], f32r, tag="xc")
                nc.sync.dma_start(xc[:], xf[:, q0 + P + 1:q0 + P + 1 + bf * CH])
            return xx, xc

        xx, xc = load_xx(0)
        pending_out = None   # (o_tile, p0) emitted one block later
        for b in range(nblocks):
            p0 = b * bf * CH
            xx_next, xc_next = load_xx(b + 1) if b + 1 < nblocks else (None, None)
            if pending_out is not None:
                po, pp0 = pending_out
                nc.sync.dma_start(
                    out[:, pp0 + P + 1:pp0 + P + 1 + bf * CH], po[:])

            o = opool.tile([C, bf * CH], f32)
            hs = []
            # phase 1: all tap matmuls (PE) + relu (ACT) — keeps the PE
            # queue free of instructions that wait on other engines.
            # chunks are processed in pairs sharing a 2-bank PSUM tile so
            # one ACT relu covers 1024 columns.
            for sp in range(bf // 2):
                ph = ph_pool.tile([HID, 2 * CH], f32)
                ntap = 1 if probe == 'taps1' else 3
                for s2 in range(2):
                    f0 = (2 * sp + s2) * CH
                    for dy in range(ntap):
                        nc.tensor.matmul(
                            ph[:, s2 * CH:(s2 + 1) * CH],
                            lhsT=wc_sb[:, dy * HID:(dy + 1) * HID],
                            rhs=xx[:, f0 + dy * P:f0 + dy * P + CH],
                            start=(dy == 0), stop=(dy == ntap - 1),
                        )
                h = hpool.tile([HID, 2 * CH], mdt)
                nc.scalar.activation(h[:], ph[:], Relu, bias=b1_sb[:])
                hs.append(h)
            # phase 2: MLP2 matmuls (PE), then post-ops (DVE)
            pdxs = []
            for s in range(bf):
                pdx = pdx_pool.tile([32, CH], f32)
                nc.tensor.matmul(pdx[:], lhsT=w2_sb[:],
                                 rhs=hs[s // 2][:, (s % 2) * CH:(s % 2 + 1) * CH],
                                 start=True, stop=True)
                pdxs.append(pdx)
            if mode == 'full':
                # u = dx + x_center per chunk, then block-wide
                # o = min(max(u + b2, 0), 1) in two fused DVE ops
                u = upool.tile([C, bf * CH], f32)
                for s in range(bf):
                    f0 = s * CH
                    if use_bf16:
                        res_src = xc[:, f0:f0 + CH].bitcast(f32)
                    else:
                        res_src = xx[0:C, f0 + P + 1:f0 + P + 1 + CH].bitcast(f32)
                    if probe == 'nott':
                        nc.vector.tensor_copy(u[:, f0:f0 + CH], pdxs[s][0:C, :])
                    else:
                        nc.vector.tensor_tensor(
                            u[:, f0:f0 + CH], pdxs[s][0:C, :], res_src, op=add)
                nc.vector.tensor_scalar(u[:], u[:], b2_sb[:], 0.0,
                                        op0=add, op1=op_max)
                nc.vector.tensor_scalar_min(o[:], u[:], 1.0)

            pending_out = (o, p0)
            xx, xc = xx_next, xc_next
        po, pp0 = pending_out
        nc.sync.dma_start(out[:, pp0 + P + 1:pp0 + P + 1 + bf * CH], po[:])
        if rep_cm is not None:
            rep_cm.__exit__(None, None, None)

    nc.compile()
    return nc


def _prep_weights(pk, W1):
    # pk [3(dy),3(dx),3(k)]; W1 [48,128] rows indexed 3*ci+k
    W1r = W1.reshape(C, 3, HID)                      # [ci, k, hid]
    Wfull = np.einsum("ydk,ckh->ydch", pk, W1r)      # [dy, dx, ci, hid]
    return np.ascontiguousarray(
        np.concatenate([Wfull[0].reshape(3 * C, HID),
                        Wfull[1].reshape(3 * C, HID),
                        Wfull[2].reshape(3 * C, HID)], axis=1), dtype=np.float32)


def kernel(x, perception_kernel, W1, b1, W2, b2):
    x = np.asarray(x, dtype=np.float32)
    pk = np.asarray(perception_kernel, dtype=np.float32)
    W1 = np.asarray(W1, dtype=np.float32)
    b1 = np.asarray(b1, dtype=np.float32)
    W2 = np.asarray(W2, dtype=np.float32)
    b2 = np.asarray(b2, dtype=np.float32)

    if "nc" not in _CACHE:
        _CACHE["nc"] = _build_program(use_bf16=True)
    nc = _CACHE["nc"]

    wc_np = _prep_weights(pk, W1)
    w2_np = np.zeros((HID, 32), np.float32)
    w2_np[:, :C] = W2
    b1_np = np.ascontiguousarray(b1.reshape(HID, 1))
    b2s = np.ascontiguousarray(b2.reshape(C, 1))

    import ml_dtypes
    bf16 = ml_dtypes.bfloat16
    in_maps = []
    for c in range(N_CORES):
        xt = np.ascontiguousarray(x[c].transpose(2, 0, 1))      # [C, S, S]
        xt = np.pad(xt, ((0, 0), (1, 1), (1, 1)), mode="wrap")  # [C, 258, 258]
        xflat = np.zeros((C, XLEN), np.float32)
        xflat[:, :FLAT] = xt.reshape(C, FLAT)
        in_maps.append({
            "xf": xflat, "xfb": xflat.astype(bf16),
            "wc": wc_np.astype(bf16), "w2": w2_np.astype(bf16),
            "b1": b1_np, "b2s": b2s,
        })

    res = run_bass_kernel_spmd(nc, in_maps, list(range(N_CORES)))
    _CACHE["exec_time_ns"] = getattr(res, "exec_time_ns", None)
    _CACHE["trace"] = getattr(res, "instructions_and_trace", None)
    outs = []
    for c in range(N_CORES):
        of = res.results[c]["out"][:, :FLAT].reshape(C, P, P)
        outs.append(of[:, 1:S + 1, 1:S + 1].transpose(1, 2, 0))
    return np.ascontiguousarray(np.stack(outs, axis=0), dtype=np.float32)



# revision 15
# speedup vs baseline: 1.4650x; 1.2399x over previous
"""Trainium2 Bass kernel for the CellularAutomata step (dense_cnn).

Math (per pixel): s = depthwise3x3(wrap_pad(x), [identity, sobel_x, sobel_y]);
h = relu(s @ W1 + b1); out = clip(x + h @ W2 + b2, 0, 1).

Strategy (pure data parallel, batch -> 8 cores, weights replicated):
  - The 3x3 conv + W1 are folded host-side into 9 effective [16,128] tap
    weights.  All 9 taps (K=144, padded to 160) run as ONE fp8-e4m3
    DoubleRow matmul per 512-pixel chunk (256 PE cycles): partitions hold
    tap-pairs (5 pairs x 16 ch = 80 partitions, 2 logical K per partition,
    j-major free layout).  fp8 x/tap-weights keep rel err ~1.1e-2 < 2e-2.
  - MLP2 (h @ W2) stays bf16: M=16 matmuls write 4 consecutive chunks into
    one [128,512] PSUM tile at partition quadrants 0/32/64/96, so the
    residual+bias+clip post-ops run as three [128,512] DVE ops per 4-chunk
    group instead of per-chunk 16-partition ops.
  - Residual x and the output use host-interleaved layouts ([128, G*512]
    with partition 32*(chunk%4)+ch) so each DMA is few fat descriptors.
  - Input taps are staged per 24-chunk super-tile from a host-prepared
    pre-sliced tensor (one contiguous run per partition -> 80 descriptor
    runs per super, spread over all 16 DMA engines).
  - Relu (+b1, ->bf16) is split between the ACT and DVE engines to balance
    engine busy time; PE (taps 0.5 cyc/px + mlp2 1 cyc/px) is the roofline.
"""

import numpy as np
import ml_dtypes
from contextlib import ExitStack

import concourse.bass as bass
import concourse.tile as tile
from concourse import bacc, mybir
from concourse.bass_utils import run_bass_kernel_spmd

B, S, C, HID = 8, 256, 16, 128
N_CORES = 8
P = S + 2                     # padded row stride = 258
FLAT = P * P                  # 66564
CH = 512                      # pixels per chunk
NCH = 132                     # chunks computed (130 real + 2 junk) = 33 groups
NGRP = NCH // 4               # 33 four-chunk groups
SUPS = [24, 24, 24, 24, 24, 12]   # chunks per super-tile
LRS = 24 * CH + 2 * P + 12         # super free length (12816, mult of 16)
NSUP = len(SUPS)
XQL = NGRP * CH               # 16896 columns of xq / out_q
NPAIR = 5                     # tap pairs (9 taps + 1 dup)
PPART = NPAIR * C             # 80 partitions for tap rhs/weights

_CACHE = {}


def _ap3(t, f0, sj, cj, sn, cn):
    """[partitions, cj, cn] AP over tile t with free strides sj/sn at
    element offset f0."""
    a = t[:]
    return bass.AP(tensor=a.tensor, offset=a.offset + f0,
                   ap=[list(a.ap[0]), [sj, cj], [sn, cn]])


def _gap_ap(t, cols, f0=0):
    """AP over [128, cols] tile covering partitions 32*s + c (s<4, c<16).

    Channel-outer iteration order so the paired DRAM AP has a 16-wide
    outermost dim (descriptors spread over 16 DMA engines)."""
    a = t[:]
    return bass.AP(tensor=a.tensor, offset=a.offset + f0,
                   ap=[[1, 16], [32, 4], [1, cols]])


def _build_program(relu_dve=1):
    f32 = mybir.dt.float32
    bf16 = mybir.dt.bfloat16
    f8 = mybir.dt.float8e4
    DR = mybir.MatmulPerfMode.DoubleRow
    Relu = mybir.ActivationFunctionType.Relu
    add = mybir.AluOpType.add
    op_max = mybir.AluOpType.max

    nc = bacc.Bacc("TRN2", target_bir_lowering=False, debug=False,
                   num_devices=N_CORES)

    xsup = nc.dram_tensor("xsup", [NSUP, PPART, 2 * LRS], f8,
                          kind="ExternalInput").ap()
    xq = nc.dram_tensor("xq", [128, XQL], f32, kind="ExternalInput").ap()
    wp = nc.dram_tensor("wp", [PPART, 2 * HID], f8, kind="ExternalInput").ap()
    w2 = nc.dram_tensor("w2", [HID, C], bf16, kind="ExternalInput").ap()
    b1 = nc.dram_tensor("b1", [HID, 1], f32, kind="ExternalInput").ap()
    b2r = nc.dram_tensor("b2r", [128, 1], f32, kind="ExternalInput").ap()
    outq = nc.dram_tensor("outq", [128, XQL], f32, kind="ExternalOutput").ap()

    with tile.TileContext(nc) as tc, ExitStack() as ctx:
        wpool = ctx.enter_context(tc.tile_pool(name="wts", bufs=1))
        wp_sb = wpool.tile([PPART, 2 * HID], f8)
        nc.sync.dma_start(wp_sb[:], wp)
        w2_sb = wpool.tile([HID, C], bf16)
        nc.sync.dma_start(w2_sb[:], w2)
        b1_sb = wpool.tile([HID, 1], f32)
        nc.sync.dma_start(b1_sb[:], b1)
        b2_sb = wpool.tile([128, 1], f32)
        nc.sync.dma_start(b2_sb[:], b2r)
        xq_sb = wpool.tile([128, XQL], f32)
        for half in range(2):
            hc = XQL // 2
            nc.sync.dma_start(xq_sb[:, half * hc:(half + 1) * hc],
                              xq[:, half * hc:(half + 1) * hc])

        xxpool = ctx.enter_context(tc.tile_pool(name="xx", bufs=2))
        hpool = ctx.enter_context(tc.tile_pool(name="h", bufs=6))
        upool = ctx.enter_context(tc.tile_pool(name="u", bufs=3))
        vpool = ctx.enter_context(tc.tile_pool(name="v", bufs=3))
        opool = ctx.enter_context(tc.tile_pool(name="o", bufs=2))
        phpool = ctx.enter_context(tc.tile_pool(name="ph", bufs=4, space="PSUM"))
        pdxpool = ctx.enter_context(tc.tile_pool(name="pdx", bufs=3, space="PSUM"))

        lhsT_tap = _ap3(wp_sb, 0, HID, 2, 1, HID)

        def load_xx(sup):
            xx = xxpool.tile([PPART, 2 * LRS], f8, tag="xx")
            src = xsup[sup]
            nc.sync.dma_start(xx[:], src)
            return xx

        xx = load_xx(0)
        pending = None   # (o_tile, col0, cols)
        g_global = 0
        for sup in range(NSUP):
            nch = SUPS[sup]
            ngrp = nch // 4
            xx_next = load_xx(sup + 1) if sup + 1 < NSUP else None
            o_sup = opool.tile([128, ngrp * CH], f32, tag="o")
            if pending is not None:
                po, pc0, pcols = pending
                nc.sync.dma_start(outq[:, pc0:pc0 + pcols], po[:])
            for g in range(ngrp):
                hs = []
                for s in range(4):
                    f0 = (g * 4 + s) * CH
                    ph = phpool.tile([HID, CH], f32)
                    nc.tensor.matmul(
                        ph[:], lhsT=lhsT_tap,
                        rhs=_ap3(xx, f0, LRS, 2, 1, CH),
                        start=True, stop=True, perf_mode=DR)
                    h = hpool.tile([HID, CH], bf16)
                    if s < relu_dve:
                        nc.vector.tensor_scalar(h[:], ph[:], b1_sb[:], 0.0,
                                                op0=add, op1=op_max)
                    else:
                        nc.scalar.activation(h[:], ph[:], Relu, bias=b1_sb[:])
                    hs.append(h)
                pdx = pdxpool.tile([128, CH], f32)
                for s in range(4):
                    nc.tensor.matmul(pdx[32 * s:32 * s + C, :],
                                     lhsT=w2_sb[:], rhs=hs[s][:],
                                     start=True, stop=True,
                                     tile_position=(0, 32 * s))
                u = upool.tile([128, CH], f32)
                nc.vector.tensor_tensor(
                    u[:], pdx[:], xq_sb[:, g_global * CH:(g_global + 1) * CH],
                    op=add)
                v = vpool.tile([128, CH], f32)
                nc.vector.tensor_scalar(v[:], u[:], b2_sb[:], 0.0,
                                        op0=add, op1=op_max)
                nc.vector.tensor_scalar_min(o_sup[:, g * CH:(g + 1) * CH],
                                            v[:], 1.0)
                g_global += 1
            pending = (o_sup, (g_global - ngrp) * CH, ngrp * CH)
            xx = xx_next
        po, pc0, pcols = pending
        nc.sync.dma_start(outq[:, pc0:pc0 + pcols], po[:])

    nc.compile()
    return nc


def _prep_weights(pk, W1):
    # folded tap weights: Wfold[tap, ci, hid] = sum_k pk[dy,dx,k] W1[ci*3+k,:]
    W1r = W1.reshape(C, 3, HID)                       # [ci, k, hid]
    Wfold = np.einsum("ydk,ckh->ydch", pk, W1r).reshape(9, C, HID)
    # pair layout [pair*16+c, j, hid]; tap 9 (pair 4, j 1) = 0
    wp = np.zeros((NPAIR, 2, C, HID), np.float32)
    for t in range(9):
        wp[t // 2, t % 2] = Wfold[t]
    wp = wp.transpose(0, 2, 1, 3).reshape(PPART, 2 * HID)  # j-major free
    return np.ascontiguousarray(wp)


def kernel(x, perception_kernel, W1, b1, W2, b2):
    x = np.asarray(x, dtype=np.float32)
    pk = np.asarray(perception_kernel, dtype=np.float32)
    W1 = np.asarray(W1, dtype=np.float32)
    b1 = np.asarray(b1, dtype=np.float32)
    W2 = np.asarray(W2, dtype=np.float32)
    b2 = np.asarray(b2, dtype=np.float32)

    if "nc" not in _CACHE:
        _CACHE["nc"] = _build_program()
    nc = _CACHE["nc"]

    f8 = ml_dtypes.float8_e4m3
    bf = ml_dtypes.bfloat16
    wp_np = _prep_weights(pk, W1).astype(f8)
    w2_np = np.ascontiguousarray(W2).astype(bf)
    b1_np = np.ascontiguousarray(b1.reshape(HID, 1))
    b2r = np.zeros((128, 1), np.float32)
    for s in range(4):
        b2r[32 * s:32 * s + C, 0] = b2

    # tap shifts for tap index t = 3*dy + dx
    shifts = [dy * P + dx for dy in range(3) for dx in range(3)] + [2 * P + 2]

    XEXT = (NSUP - 1) * 24 * CH + LRS + 2 * P + 4
    in_maps = []
    for c in range(N_CORES):
        xt = np.ascontiguousarray(x[c].transpose(2, 0, 1))      # [C, S, S]
        xt = np.pad(xt, ((0, 0), (1, 1), (1, 1)), mode="wrap")  # [C, 258, 258]
        xflat = np.zeros((C, XEXT), np.float32)
        xflat[:, :FLAT] = xt.reshape(C, FLAT)
        x8 = xflat.astype(f8)

        # xsup [NSUP, 80, 2, LRS]: partition pair*16+cc, j-major
        xsup = np.zeros((NSUP, NPAIR, C, 2, LRS), f8)
        for sup in range(NSUP):
            q0 = sup * 24 * CH
            for t in range(10):
                sh = shifts[t]
                xsup[sup, t // 2, :, t % 2, :] = x8[:, q0 + sh:q0 + sh + LRS]
        xsup = xsup.reshape(NSUP, PPART, 2 * LRS)

        # xq [128, XQL] f32: xq[32*s+cc, g*512+n] = xflat[cc, (4g+s)*512+P+1+n]
        xc = np.zeros((C, NCH * CH), np.float32)
        avail = min(NCH * CH, XEXT - (P + 1))
        xc[:, :avail] = xflat[:, P + 1:P + 1 + avail]
        xq_np = np.zeros((128, XQL), np.float32)
        xcr = xc.reshape(C, NGRP, 4, CH)
        for s in range(4):
            xq_np[32 * s:32 * s + C, :] = xcr[:, :, s, :].reshape(C, XQL)

        in_maps.append({
            "xsup": xsup, "xq": xq_np, "wp": wp_np, "w2": w2_np,
            "b1": b1_np, "b2r": b2r,
        })

    res = run_bass_kernel_spmd(nc, in_maps, list(range(N_CORES)))
    _CACHE["exec_time_ns"] = getattr(res, "exec_time_ns", None)
    _CACHE["trace"] = getattr(res, "instructions_and_trace", None)

    outs = []
    for c in range(N_CORES):
        oq = res.results[c]["outq"]                  # [128, XQL]
        # invert: of[cc, (4g+s)*512+n] = oq[32s+cc, g*512+n]
        of = np.empty((C, NGRP, 4, CH), np.float32)
        for s in range(4):
            of[:, :, s, :] = oq[32 * s:32 * s + C].reshape(C, NGRP, CH)
        of = of.reshape(C, NCH * CH)[:, :FLAT - (P + 1)]
        full = np.zeros((C, FLAT), np.float32)
        full[:, P + 1:] = of
        img = full.reshape(C, P, P)[:, 1:S + 1, 1:S + 1]
        outs.append(img.transpose(1, 2, 0))
    return np.ascontiguousarray(np.stack(outs, axis=0), dtype=np.float32)
